# revision 11
# baseline (speedup 1.0000x reference)
"""AttentionLayer Trainium2 kernel: 8-way SPMD (batch x query-half data parallel).

Per core (b = core//2, h = core%2), with x rotated so the core's query half
occupies columns 0..2047:
  k  = wk @ x + bk            [32, 4096]
  q  = wq @ x[:, :2048] + bq  [32, 2048]
  vT = x^T @ wv^T             [4096, 256]   (v transposed, born in [j, c] layout)
  S^T[j, i] = k[:, j]^T q[:, i]   -> P = exp(S^T - 8)  (fixed shift keeps P in
                                     fp8e5 range; shift cancels in the ratio)
  out[i, c] = (sum_j P[j, i] vT[j, c]) / (sum_j P[j, i]) + x^T[i, c]

PV is TRANSPOSED relative to the obvious mapping: P (fp8e5) is the
*stationary* operand ([j, 2, i-chunk] DoubleRow tiles) and vT (fp8e4,
byte-interleaved pairs) is the *moving* operand, so the output lands as
[i-part, c-free].  Two payoffs:
  - the moving vT carries a 257th all-ones column, so the softmax
    denominator accumulates as pv column 256 of the same matmul -- no
    separate denominator matmuls (the baseline spent ~14us of PE on them);
  - denominators come out per-*partition* ([128,1]), so the final divide is
    a cheap tensor_scalar_mul -- no ones-row broadcast matmul, no f32r cast.
Output is written i-major ([NQ, C]) and transposed on the host.

fp8 DoubleRow elsewhere: vT projection uses a host-uploaded fp8e4 copy of x
(stationary, ci-block pair) against byte-interleaved fp8 wv^T -- identical
matmul geometry to the new PV, which is why the layout is trusted.

Scores PSUM is split into two [128, 1024] ping-pong halves (scA/scB) so the
next block's score matmuls overlap the current block's exp.  ACT (exp at
~1113ns per [128,1024] half, 64 halves) is the pacing engine; everything
else -- projections, vT, PV -- is arranged to keep it saturated from ~13us.
All projections and all 16 vT pairs run in the prologue in program order
(no priority hoisting: a hoisted vT matmul waiting on the xf8 DMA would
head-of-line block the score matmuls behind it).
"""
import numpy as np
import ml_dtypes

import concourse.bacc as bacc
import concourse.tile as tile
from concourse import mybir
from concourse.bass_utils import run_bass_kernel_spmd

F32 = mybir.dt.float32
BF16 = mybir.dt.bfloat16
F8P = mybir.dt.float8e5    # P = exp(scores - OFF): wide range, 2-bit mantissa
F8V = mybir.dt.float8e4    # vT / x / wv: |.| <~ 6, fine mantissa
AF = mybir.ActivationFunctionType
ALU = mybir.AluOpType
DR = mybir.MatmulPerfMode.DoubleRow

C = 256          # channels
OC = C + 1       # pv output columns: 256 channels + denominator ones column
VS = 2 * OC      # vt bytes per pair slot (interleaved pairs + ones pair)
D = 32           # q/k dim (C // 8)
N = 4096         # h*w
NQ = 2048        # queries per core
NCORE = 8
NG = 8           # score groups per slice (4 j-blocks each)
OFF = 8.0        # exp shift: max score ~13.1 -> max P ~ e^5.1 ~ 158 (fp8e5 ok)

_cache = {}


PRE = 7          # prologue-pre-run score blocks: ACT runway that covers the
                 # vt pipeline + the pvt-bank WAR on the last vt evacuations


def _build():
    nc = bacc.Bacc(None, target_bir_lowering=False)
    xb_ext = nc.declare_dram_parameter("xb", [C, N], BF16, isOutput=False)
    xres_ext = nc.declare_dram_parameter("xres", [NQ, C], F32, isOutput=False)
    wqt_ext = nc.declare_dram_parameter("wqt", [C, D], BF16, isOutput=False)
    wkt_ext = nc.declare_dram_parameter("wkt", [C, D], BF16, isOutput=False)
    wv8_ext = nc.declare_dram_parameter("wv8", [128, 2 * C], F8V,
                                        isOutput=False)
    bq4_ext = nc.declare_dram_parameter("bq4", [128, 1], F32, isOutput=False)
    bk4_ext = nc.declare_dram_parameter("bk4", [128, 1], F32, isOutput=False)
    out_ext = nc.declare_dram_parameter("out", [NQ, C], F32, isOutput=True)

    with tile.TileContext(nc) as tc:
        with (
            tc.tile_pool(name="const", bufs=1) as const,
            tc.tile_pool(name="big", bufs=1) as big,
            tc.tile_pool(name="pbuf", bufs=PRE + 3) as pbuf,
            tc.tile_pool(name="work", bufs=3) as work,
            tc.tile_pool(name="ps_scA", bufs=1, space="PSUM") as ps_scA,
            tc.tile_pool(name="ps_scB", bufs=1, space="PSUM") as ps_scB,
            tc.tile_pool(name="ps_pv", bufs=1, space="PSUM") as ps_pv,
        ):
            wqt_sb = const.tile([128, 2 * D], BF16)
            wkt_sb = const.tile([128, 2 * D], BF16)
            wv8_sb = const.tile([128, 2 * C], F8V)   # [p, 2c+u], u=ci block
            bq4_sb = const.tile([128, 1], F32)
            bk4_sb = const.tile([128, 1], F32)
            negoff = const.tile([128, 1], F32)

            x_sb = big.tile([128, 2 * N], BF16)       # ci blocks side by side
            xf8_sb = big.tile([128, 2 * N], F8V)      # fp8 copy for vT proj
            xres_sb = big.tile([128, 16 * C], F32)    # [i%128, (4t+k)*C + c]
            # k4: strip r (partitions 32r..32r+31) holds j-blocks 4g+r at
            # free g*128..(g+1)*128
            k4_sb = big.tile([128, 1024], BF16)
            # q4: strip r holds a full copy of q (slices side by side)
            q4_sb = big.tile([128, NQ], BF16)
            # vt: pair slot s (j-blocks 2s, 2s+1) at free s*VS, interleaved
            # [p, 2c+u]; columns 2C..2C+1 are the all-ones denominator pair
            vt_sb = big.tile([128, 16 * VS], F8V)

            # DMA issue costs ~650ns of sequencer time per dma_start, so use
            # few multi-dim descriptors.  x is split over both queues; the
            # fp8 copy of x is cast on the (idle) GpSimd engine instead of
            # being a second 1MB upload.  xres comes in 512-row chunks, two
            # per queue, behind x -- chunk t is needed at epilogue_b(t).
            x3_sb = x_sb[:].rearrange("p (two n) -> p two n", two=2)
            x3_ext = xb_ext[:].rearrange("(two p) n -> p two n", two=2)
            xf3_sb = xf8_sb[:].rearrange("p (two n) -> p two n", two=2)
            xres3_sb = xres_sb[:].rearrange("p (n c) -> p n c", c=C)
            xres3_ext = xres_ext[:].rearrange("(n p) c -> p n c", p=128)
            nc.scalar.dma_start(
                wqt_sb[:].rearrange("p (two d) -> p two d", two=2),
                wqt_ext[:].rearrange("(two p) d -> p two d", two=2))
            nc.scalar.dma_start(x3_sb[:, :, 0:512], x3_ext[:, :, 0:512])
            nc.scalar.dma_start(x3_sb[:, :, 512:1024], x3_ext[:, :, 512:1024])
            nc.scalar.dma_start(x3_sb[:, :, 1024:2048],
                                x3_ext[:, :, 1024:2048])
            nc.scalar.dma_start(xres3_sb[:, 0:4, :], xres3_ext[:, 0:4, :])
            nc.scalar.dma_start(xres3_sb[:, 4:8, :], xres3_ext[:, 4:8, :])
            nc.sync.dma_start(
                wkt_sb[:].rearrange("p (two d) -> p two d", two=2),
                wkt_ext[:].rearrange("(two p) d -> p two d", two=2))
            nc.sync.dma_start(bq4_sb[:], bq4_ext[:])
            nc.sync.dma_start(bk4_sb[:], bk4_ext[:])
            nc.sync.dma_start(wv8_sb[:], wv8_ext[:])
            nc.sync.dma_start(x3_sb[:, :, 2048:3072], x3_ext[:, :, 2048:3072])
            nc.sync.dma_start(x3_sb[:, :, 3072:4096], x3_ext[:, :, 3072:4096])
            nc.sync.dma_start(xres3_sb[:, 8:12, :], xres3_ext[:, 8:12, :])
            nc.sync.dma_start(xres3_sb[:, 12:16, :], xres3_ext[:, 12:16, :])
            # fp8 x chunks chase the bf16 x DMA on GpSimd
            for lo, hi in ((0, 512), (512, 1024), (1024, 2048),
                           (2048, 3072), (3072, 4096)):
                nc.gpsimd.tensor_copy(xf3_sb[:, :, lo:hi],
                                      x3_sb[:, :, lo:hi])
            nc.vector.memset(negoff[:], -OFF)
            # all-ones vt columns (denominator): one strided memset
            ones_view = vt_sb[:].rearrange("p (s r) -> p s r",
                                           r=VS)[:, :, 2 * C:VS]
            nc.vector.memset(ones_view, 1.0)

            def k_proj0_half(h, tag):
                """k4 for g in (2h, 2h+1): j-blocks 8h..8h+7, needs x columns
                [1024h, 1024h+1024) only -- chases the x DMA."""
                ps = ps_pv.tile([128, 512], F32, tag=tag, name="k_ps")
                for r in range(4):
                    for ci in range(2):
                        base = ci * N + (8 * h + r) * 128
                        rhs = x_sb[:, base: base + 5 * 128]
                        rhs = rhs.rearrange("p (g f) -> p g f", f=128)[:, 0:5:4, :]
                        nc.tensor.matmul(
                            ps[32 * r:32 * (r + 1), 0:256],
                            wkt_sb[:, ci * D:(ci + 1) * D],
                            rhs,
                            start=(ci == 0), stop=(ci == 1),
                            tile_position=(0, 32 * r))
                nc.vector.tensor_scalar_add(
                    k4_sb[:, h * 256:(h + 1) * 256], ps[:, 0:256], bk4_sb[:])

            def k_proj(gh, tag):
                """Fill k4_sb[:, gh*512:(gh+1)*512] (j-blocks 16gh..16gh+15)."""
                ps = ps_pv.tile([128, 512], F32, tag=tag, name="k_ps")
                for r in range(4):
                    for ci in range(2):
                        base = ci * N + (16 * gh + r) * 128
                        rhs = x_sb[:, base: base + 13 * 128]
                        rhs = rhs.rearrange("p (g f) -> p g f", f=128)[:, 0:13:4, :]
                        nc.tensor.matmul(
                            ps[32 * r:32 * (r + 1), :],
                            wkt_sb[:, ci * D:(ci + 1) * D],
                            rhs,
                            start=(ci == 0), stop=(ci == 1),
                            tile_position=(0, 32 * r))
                nc.vector.tensor_scalar_add(
                    k4_sb[:, gh * 512:(gh + 1) * 512], ps[:], bk4_sb[:])

            def q_proj(t, tag):
                """Fill q4_sb[:, t*512:(t+1)*512]: q slice replicated in 4 strips."""
                ps = ps_pv.tile([128, 512], F32, tag=tag, name="q_ps")
                for r in range(4):
                    for ci in range(2):
                        nc.tensor.matmul(
                            ps[32 * r:32 * (r + 1), :],
                            wqt_sb[:, ci * D:(ci + 1) * D],
                            x_sb[:, ci * N + t * 512: ci * N + (t + 1) * 512],
                            start=(ci == 0), stop=(ci == 1),
                            tile_position=(0, 32 * r))
                nc.vector.tensor_scalar_add(
                    q4_sb[:, t * 512:(t + 1) * 512], ps[:], bq4_sb[:])

            xf8_3d = xf8_sb[:].rearrange("p (two n) -> p two n", two=2)
            wv8_3d = wv8_sb[:].rearrange("p (c two) -> p two c", two=2)

            def vt_proj_pair(jb, tag):
                """vT for j-blocks jb and jb+1: one fp8 DR matmul per j-block
                (contraction = 256 channels as (ci-block pair) x partition),
                one PSUM bank; evacuation writes byte-interleaved pairs."""
                vps = ps_pv.tile([128, 2 * C], F32, tag=tag, name="vt_ps")
                for u in range(2):
                    nc.tensor.matmul(
                        vps[:, u * C:(u + 1) * C],
                        xf8_3d[:, :, (jb + u) * 128:(jb + u + 1) * 128],
                        wv8_3d,
                        start=True, stop=True,
                        perf_mode=DR)
                dst = vt_sb[:, (jb // 2) * VS: (jb // 2) * VS + 2 * C]
                nc.vector.tensor_copy(
                    dst.rearrange("p (c two) -> p two c", two=2),
                    vps[:].rearrange("p (two c) -> p two c", two=2))

            pairs = [(t, g) for t in range(4) for g in range(NG)]
            pvt = {}
            rinvs = {}
            p_tiles = {}

            def scores_half(t, g, half):
                """Score matmuls for strips (2*half, 2*half+1) into a 2-bank
                PSUM tile, then exp into the fp8 P tile; half h occupies
                [p, h*1024 + u*512 + i] (u = strip in half) -- the natural
                stationary layout for the transposed PV."""
                pool = ps_scA if half == 0 else ps_scB
                sc = pool.tile([128, 1024], F32, tag=f"sc{half}",
                               name=f"sc{half}")
                for rr in range(2):
                    r = 2 * half + rr
                    nc.tensor.matmul(
                        sc[:, rr * 512:(rr + 1) * 512],
                        k4_sb[32 * r:32 * (r + 1), g * 128:(g + 1) * 128],
                        q4_sb[32 * r:32 * (r + 1), t * 512:(t + 1) * 512],
                        start=True, stop=True,
                        tile_position=(32 * r, 0))
                p_sb = p_tiles[(t, g)]
                nc.scalar.activation(
                    p_sb[:, half * 1024:(half + 1) * 1024], sc[:],
                    AF.Exp, bias=negoff[:])

            def pv_pair(t, g, h):
                """Transposed PV for pair slot 2g+h (j-blocks 4g+2h, 4g+2h+1):
                P is stationary ([j, 2, 128i] per i-chunk), vt+ones moving;
                accumulates out[i, c] and (column 256) the denominator."""
                p_sb = p_tiles[(t, g)]
                P3 = p_sb[:, h * 1024:(h + 1) * 1024].rearrange(
                    "p (two i) -> p two i", two=2)
                slot = 2 * g + h
                vt3 = vt_sb[:, slot * VS: (slot + 1) * VS].rearrange(
                    "p (c two) -> p two c", two=2)
                first = (g == 0 and h == 0)
                last = (g == NG - 1 and h == 1)
                for k in range(4):
                    nc.tensor.matmul(
                        pvt[t][k][:],
                        P3[:, :, k * 128:(k + 1) * 128],
                        vt3,
                        start=first, stop=last,
                        perf_mode=DR)

            def epilogue_a(t):
                """After the last PV of slice t: per-partition 1/denominator."""
                rs = []
                for k in range(4):
                    rinv = work.tile([128, 1], F32, tag=f"rinv{k}",
                                     name=f"rinv{k}")
                    nc.vector.reciprocal_approx_fast(
                        rinv[:], pvt[t][k][:, C:C + 1])
                    rs.append(rinv)
                rinvs[t] = rs

            def epilogue_b(t):
                """Divide + residual + store; i-tiles split Vector/GpSimd so
                the final slice's epilogue chain halves in wall time."""
                rs = rinvs.pop(t)
                for k in range(4):
                    # GpSimd cannot read PSUM: the divide (PSUM source) is
                    # always on Vector; the SBUF-only residual add of odd
                    # tiles goes to GpSimd.
                    eng, sfx = ((nc.vector, "v") if k % 2 == 0
                                else (nc.gpsimd, "g"))
                    o_tmp = work.tile([128, C], F32, tag=f"o_tmp{sfx}",
                                      name="o_tmp")
                    nc.vector.tensor_scalar_mul(
                        o_tmp[:], pvt[t][k][:, 0:C], rs[k][:])
                    o_out = work.tile([128, C], F32, tag=f"o_out{sfx}",
                                      name="o_out")
                    eng.tensor_add(
                        o_out[:], o_tmp[:],
                        xres_sb[:, (4 * t + k) * C: (4 * t + k + 1) * C])
                    nc.sync.dma_start(
                        out_ext[(4 * t + k) * 128:(4 * t + k + 1) * 128, :],
                        o_out[:])

            def stage1(t, g):
                p_tiles[(t, g)] = pbuf.tile([128, 2048], F8P, tag="p",
                                            name="p_sb")
                scores_half(t, g, 0)
                scores_half(t, g, 1)

            # ---- prologue: chase the x DMA with the projections and
            # pre-run PRE blocks of scores+exp.  The pre-run is the ACT
            # runway that covers the whole vt pipeline: the loop's first PV
            # matmuls WAR-wait on the last vt evacuations (they share the
            # pv0..pv3 PSUM tags), stalling the in-order Tensor queue until
            # ~28us -- by which time ACT is still busy with pre-run exps.
            q_proj(0, "pv0")
            k_proj0_half(0, "pv1")
            stage1(0, 0)
            k_proj0_half(1, "pv2")
            stage1(0, 1)
            q_proj(1, "pv3")
            stage1(0, 2)
            stage1(0, 3)
            q_proj(2, "pv0")
            q_proj(3, "pv1")
            k_proj(1, "pv2")
            stage1(0, 4)
            stage1(0, 5)
            stage1(0, 6)
            for idx, jb in enumerate(range(0, 32, 2)):
                vt_proj_pair(jb, f"pv{(3 + idx) % 4}")

            # ---- main loop: stage1 (scores+exp) runs 2 blocks ahead of PV
            # (first 4 blocks pre-run above).  Scores issue first each
            # iteration (their WAR on the 2-blocks-ago exp cleared long ago)
            # so ACT never starves; the 8 PV matmuls follow.
            for i in range(1, len(pairs) + 2):
                cur = pairs[i + 1] if i + 1 < len(pairs) else None
                if cur is not None and i + 1 < PRE:
                    cur = None      # pre-run in the prologue
                prev = pairs[i - 1] if i <= len(pairs) else None

                if prev is not None and prev[1] == 0:
                    pvt[prev[0]] = [
                        ps_pv.tile([128, OC], F32, tag=f"pv{k}",
                                   name=f"pv{k}")
                        for k in range(4)]
                if cur is not None:
                    p_tiles[cur] = pbuf.tile([128, 2048], F8P, tag="p",
                                             name="p_sb")
                    scores_half(*cur, 0)
                    scores_half(*cur, 1)
                if prev is not None:
                    pv_pair(*prev, 0)
                    pv_pair(*prev, 1)
                if prev is not None and prev[1] == NG - 1:
                    epilogue_a(prev[0])
                if i >= 2 and i - 2 < len(pairs):
                    tq, gq = pairs[i - 2]
                    if gq == NG - 1:
                        epilogue_b(tq)
    nc.compile()
    return nc


def _get_nc():
    if "nc" not in _cache:
        _cache["nc"] = _build()
    return _cache["nc"]


def _in_maps(x, wq, bq, wk, bk, wv, bv):
    wqt = np.ascontiguousarray(wq.T).astype(ml_dtypes.bfloat16)
    wkt = np.ascontiguousarray(wk.T).astype(ml_dtypes.bfloat16)
    # wv8[p, 2c+u] = wv[c, 128u+p] (byte-interleaved ci-block pairs)
    wv8 = np.ascontiguousarray(
        wv.T.reshape(2, 128, C).transpose(1, 2, 0).reshape(128, 2 * C)
    ).astype(ml_dtypes.float8_e4m3)
    bq4 = np.ascontiguousarray(
        np.tile(np.asarray(bq, np.float32).reshape(D, 1), (4, 1)))
    bk4 = np.ascontiguousarray(
        np.tile(np.asarray(bk, np.float32).reshape(D, 1), (4, 1)))
    maps = []
    for core in range(NCORE):
        b, h = core // 2, core % 2
        xb = np.asarray(x[b], dtype=np.float32).reshape(C, N)
        if h == 1:
            xc = np.concatenate([xb[:, NQ:], xb[:, :NQ]], axis=1)
        else:
            xc = xb
        xb16 = np.ascontiguousarray(xc).astype(ml_dtypes.bfloat16)
        maps.append({
            "xb": xb16,
            "xres": np.ascontiguousarray(
                xc[:, :NQ].T + np.asarray(bv, np.float32)[None, :]),
            "wqt": wqt, "wkt": wkt, "wv8": wv8,
            "bq4": bq4, "bk4": bk4,
        })
    return maps


def _get_runner():
    """Build the SPMD graph once and cache a reusable jitted executable
    (run_bass_kernel_spmd re-jits per call, paying a full XLA compile)."""
    if "runner" in _cache:
        return _cache["runner"]
    import jax
    from jax.sharding import Mesh, PartitionSpec
    from jax.experimental.shard_map import shard_map
    from concourse import bass2jax, mybir as mb

    nc = _get_nc()
    bass2jax.install_neuronx_cc_hook()
    partition_name = (nc.partition_id_tensor.name
                      if nc.partition_id_tensor else None)
    in_names, out_names, out_avals, zero_shapes = [], [], [], []
    for alloc in nc.m.functions[0].allocations:
        if not isinstance(alloc, mb.MemoryLocationSet):
            continue
        name = alloc.memorylocations[0].name
        if alloc.kind == "ExternalInput":
            if name != partition_name:
                in_names.append(name)
        elif alloc.kind == "ExternalOutput":
            out_names.append(name)
            shape = tuple(alloc.tensor_shape)
            dtype = mb.dt.np(alloc.dtype)
            out_avals.append(jax.core.ShapedArray(shape, dtype))
            zero_shapes.append((shape, dtype))
    n_params = len(in_names)
    full_in_names = list(in_names) + list(out_names)
    if partition_name is not None:
        full_in_names.append(partition_name)
    donate = tuple(range(n_params, n_params + len(out_names)))

    def _body(*args):
        operands = list(args)
        if partition_name is not None:
            operands.append(bass2jax.partition_id_tensor())
        outs = bass2jax._bass_exec_p.bind(
            *operands,
            out_avals=tuple(out_avals),
            in_names=tuple(full_in_names),
            out_names=tuple(out_names),
            lowering_input_output_aliases=(),
            sim_require_finite=True,
            sim_require_nnan=True,
            nc=nc,
        )
        return tuple(outs)

    devices = jax.devices()[:NCORE]
    mesh = Mesh(np.asarray(devices), ("core",))
    in_specs = (PartitionSpec("core"),) * (n_params + len(out_names))
    out_specs = (PartitionSpec("core"),) * len(out_names)
    sharded = jax.jit(
        shard_map(_body, mesh=mesh, in_specs=in_specs, out_specs=out_specs,
                  check_rep=False),
        donate_argnums=donate, keep_unused=True)
    runner = (sharded, in_names, out_names, out_avals, zero_shapes)
    _cache["runner"] = runner
    return runner


def _run_fast(maps):
    sharded, in_names, out_names, out_avals, zero_shapes = _get_runner()
    concat_in = [
        np.concatenate([np.asarray(maps[c][name]) for c in range(NCORE)], axis=0)
        for name in in_names
    ]
    concat_zeros = [
        np.zeros((NCORE * s[0], *s[1:]), dt) for s, dt in zero_shapes
    ]
    out_arrs = sharded(*concat_in, *concat_zeros)
    return [
        {name: np.asarray(out_arrs[i]).reshape(NCORE, *out_avals[i].shape)[c]
         for i, name in enumerate(out_names)}
        for c in range(NCORE)
    ]


def _assemble(results):
    out = np.empty((4, C, N), dtype=np.float32)
    for core in range(NCORE):
        b, h = core // 2, core % 2
        out[b][:, h * NQ:(h + 1) * NQ] = results[core]["out"].T
    return out.reshape(4, C, 64, 64)


def _run(inputs, trace=False, tmpdir=None):
    maps = _in_maps(**inputs)
    if trace:
        nc = _get_nc()
        res = run_bass_kernel_spmd(nc, maps, core_ids=list(range(NCORE)),
                                   trace=trace, tmpdir=tmpdir)
        return _assemble(res.results), res
    return _assemble(_run_fast(maps)), None


def kernel(**inputs):
    out, _ = _run(inputs)
    return out
